# revision 37
# baseline (speedup 1.0000x reference)
"""BiasedMHA + GLU fused Trainium2 kernel (v2: exp-bias multiply).

Problem: out = GLU(x) + OutProj(MHA(x, attn_bias))  with
  B=8, N=1024, D=768, H=12, HD=64, fp32 inputs/outputs.

Strategy: data-parallel over batch across the 8 NeuronCores (one batch
element per core, no collectives). Everything in a "transposed"
[channel, token] layout so every GEMM contracts the partition dim.

Key changes vs v1 (286us):
  * The additive attention bias is applied as exp(s+b) = exp(s)*exp(b):
    the host precomputes exp(attn_bias) in bf16 tiled exactly like the
    scores PSUM layout [k, (kt4 q)]; after ScalarE exp of the raw qk
    scores, a single elementwise multiply (DVE for one head of the
    pair, GpSimd for the other) applies the bias. This removes the 768
    PE identity-matmuls + LDWEIGHTS (~90us of PE time) that v1 spent
    transposing/injecting the bias via the PE array.
  * All GEMMs run in bf16 (same 1 col/cycle PE rate as fp32r, but
    FWL-accelerated weight loads, half the DMA/SBUF, and 2x DVE modes).
  * K-halves accumulate into one PSUM ctx tile (K-inner loop): no
    partial-context eviction/re-inject round trip.
  * The GLU gate and out-proj GEMMs are interleaved into the attention
    epochs as PE fillers: attention is ScalarE(exp)-paced (~2us/epoch
    vs 1.3us of PE work), so phase-D work rides in the PE idle slots.
    bo is folded in via a [1,128] ones-row matmul into the same PSUM.

  Error budget: bf16 rounding of x/weights/q/k/v/exp adds ~5e-3
  relative error (vs 2e-2 tolerance), validated in numpy simulation.
"""

import os
import sys

for _p in ("/opt/trn_rl_repo", "/root/.axon_site/_ro/trn_rl_repo"):
    if os.path.isdir(_p) and _p not in sys.path:
        sys.path.insert(0, _p)

import numpy as np
import ml_dtypes

import concourse.bacc as bacc
import concourse.mybir as mybir
from concourse import tile
from concourse.bass_utils import run_bass_kernel_spmd

B, N, D, H, HD = 8, 1024, 768, 12, 64
P = 128
ND = D // P           # 6 channel tiles
NN = N // P           # 8 token tiles
VW = H * (HD + 1)     # 780: v layout [token, h*(64+1)] with ones column

F32 = mybir.dt.float32
BF16 = mybir.dt.bfloat16
AF = mybir.ActivationFunctionType
OP = mybir.AluOpType


def _bf(x):
    return np.ascontiguousarray(x, dtype=np.float32).astype(ml_dtypes.bfloat16)


def _emit(nc, tc, xT, ebd, w, bvec, outT, dbg=None):
    with tc.tile_pool(name="const", bufs=1) as constp, \
         tc.tile_pool(name="xp", bufs=1) as xp, \
         tc.tile_pool(name="qkvT", bufs=1) as qkvp, \
         tc.tile_pool(name="ctxTp", bufs=1) as ctxp, \
         tc.tile_pool(name="ebp", bufs=12) as ebp, \
         tc.tile_pool(name="esp", bufs=3) as esp, \
         tc.tile_pool(name="epi", bufs=1) as epip, \
         tc.tile_pool(name="wDE", bufs=1) as wde:

        # ---- constants + ScalarE exp-table warmup (load during phase B)
        dum = constp.tile([1, 16], F32, tag="dum", name="dum")
        nc.vector.memset(dum[:], 0.25)
        dum2 = constp.tile([1, 16], F32, tag="dum2", name="dum2")
        nc.scalar.activation(dum2[:], dum[:], AF.Exp)

        bvt = {}
        for nm in ("bq", "bk", "bgh"):
            t = constp.tile([P, ND], F32, tag=f"t{nm}", name=f"t{nm}")
            nc.sync.dma_start(t[:], bvec[nm].ap().rearrange("(j p) -> p j", p=P))
            bvt[nm] = t
        ones12 = constp.tile([P, H], BF16, tag="ones12", name="ones12")
        nc.vector.memset(ones12[:], 1.0)
        onesrow = constp.tile([1, N], BF16, tag="onesrow", name="onesrow")
        nc.vector.memset(onesrow[:], 1.0)

        xsb = [xp.tile([P, N], BF16, tag=f"x{i}", name=f"x{i}")
               for i in range(ND)]
        qT = [qkvp.tile([P, N], BF16, tag=f"qT{i}", name=f"qT{i}")
              for i in range(ND)]
        kT = [qkvp.tile([P, N], BF16, tag=f"kT{i}", name=f"kT{i}")
              for i in range(ND)]
        vsb = [qkvp.tile([P, VW], BF16, tag=f"v{t}", name=f"v{t}")
               for t in range(NN)]
        vv = [t.rearrange("p (h c) -> p h c", c=HD + 1) for t in vsb]
        ctxT = [ctxp.tile([P, N], BF16, tag=f"cT{i}", name=f"cT{i}")
                for i in range(ND)]
        u2 = [epip.tile([P, 512], BF16, tag=f"u2_{j}_{c}", name=f"u2_{j}_{c}")
              for j in range(ND) for c in range(2)]

        wo = [wde.tile([P, D], BF16, tag=f"wo{i}", name=f"wo{i}")
              for i in range(ND)]
        wg = [wde.tile([P, D], BF16, tag=f"wg{i}", name=f"wg{i}")
              for i in range(ND)]
        borow = wde.tile([1, D], BF16, tag="borow", name="borow")

        # ---------------- Phase B: q/k/v projections ----------------
        with tc.tile_pool(name="xTw", bufs=1) as xwp, \
             tc.tile_pool(name="psB", bufs=2, space="PSUM") as psB:
            wsb = {nm: [xwp.tile([P, D], BF16, tag="wpool",
                                 name=f"{nm}{i}", bufs=18)
                        for i in range(ND)]
                   for nm in ("wq", "wk", "wv")}
            # critical-path DMAs first, split so the j=0/c=0 GEMM group can
            # start after ~1.2MB instead of ~4.5MB.
            for i in range(ND):
                nc.sync.dma_start(wsb["wq"][i][:, 0:P],
                                  w["wq"][i * P:(i + 1) * P, 0:P])
                nc.sync.dma_start(xsb[i][:, 0:512],
                                  xT[i * P:(i + 1) * P, 0:512])
            for i in range(ND):
                nc.sync.dma_start(wsb["wq"][i][:, P:D],
                                  w["wq"][i * P:(i + 1) * P, P:D])
                nc.sync.dma_start(xsb[i][:, 512:N],
                                  xT[i * P:(i + 1) * P, 512:N])
            for i in range(ND):
                nc.sync.dma_start(wsb["wk"][i][:], w["wk"][i * P:(i + 1) * P, :])
            for i in range(ND):
                nc.sync.dma_start(wsb["wv"][i][:], w["wv"][i * P:(i + 1) * P, :])
            # phase C/D weights queued behind the phase-B critical path;
            # the attention-bias stream (emitted in phase C) follows them.
            for i in range(ND):
                nc.sync.dma_start(wg[i][:], w["wg"][i * P:(i + 1) * P, :])
            for i in range(ND):
                nc.sync.dma_start(wo[i][:], w["wo"][i * P:(i + 1) * P, :])
            nc.sync.dma_start(borow[:], w["borow"][0:1, :])

            # warmup burst: keeps the PE busy through the initial DMA wait
            # so the HAM clock-gate opens to 2.4 GHz before the real GEMMs.
            wup = constp.tile([P, 512], BF16, tag="wup", name="wup")
            nc.vector.memset(wup[:], 0.125)
            wps = psB.tile([P, 512], F32, tag="psB", name="psB")
            for i in range(12):
                nc.tensor.matmul(wps[:], wup[:, 0:P], wup[:],
                                 start=True, stop=True, skip_group_check=True)

            for nm, dst in (("wq", qT), ("wk", kT)):
                bt = bvt["bq" if nm == "wq" else "bk"]
                for j in range(ND):
                    for c in range(2):
                        ps = psB.tile([P, 512], F32, tag="psB", name="psB")
                        for i in range(ND):
                            nc.tensor.matmul(
                                ps[:], wsb[nm][i][:, j * P:(j + 1) * P],
                                xsb[i][:, c * 512:(c + 1) * 512],
                                start=(i == 0), stop=(i == ND - 1))
                        nc.scalar.activation(dst[j][:, c * 512:(c + 1) * 512],
                                             ps[:], AF.Identity,
                                             bias=bt[:, j:j + 1])

            for t in range(NN):
                nc.vector.tensor_copy(vv[t][:, :, HD], ones12[:])
                for c, (lo, sz) in enumerate(((0, 512), (512, 256))):
                    ps = psB.tile([P, 512], F32, tag="psB", name="psB")
                    for i in range(ND):
                        nc.tensor.matmul(
                            ps[:, 0:sz], xsb[i][:, t * P:(t + 1) * P],
                            wsb["wv"][i][:, lo:lo + sz],
                            start=(i == 0), stop=(i == ND - 1))
                    h0 = lo // HD
                    nc.vector.tensor_copy(
                        vv[t][:, h0:h0 + sz // HD, 0:HD],
                        ps[:, 0:sz].rearrange("p (h c) -> p h c", c=HD))

        if dbg is not None:
            nc.sync.dma_start(dbg["qT0"], qT[0][:])
            nc.sync.dma_start(dbg["kT0"], kT[0][:])
            nc.sync.dma_start(dbg["v0"], vsb[0][:])

        # ---------------- Phase C: attention + interleaved D fillers ----
        with tc.tile_pool(name="psS", bufs=2, space="PSUM") as psS, \
             tc.tile_pool(name="psC", bufs=2, space="PSUM") as psC, \
             tc.tile_pool(name="psD", bufs=2, space="PSUM") as psD, \
             tc.tile_pool(name="norm", bufs=2) as normp, \
             tc.tile_pool(name="finp", bufs=4) as finp:

            def emit_gate(jc):
                j, c = jc // 2, jc % 2
                sl = slice(c * 512, (c + 1) * 512)
                pg = psD.tile([P, 512], F32, tag="psD", name="psD")
                for i in range(ND):
                    nc.tensor.matmul(pg[:], wg[i][:, j * P:(j + 1) * P],
                                     xsb[i][:, sl],
                                     start=(i == 0), stop=(i == ND - 1))
                th = normp.tile([P, 512], BF16, tag="th", name="th", bufs=3)
                # tanh(0.5*g + 0.5*bg); bgh = 0.5*bg host-side
                nc.scalar.activation(th[:], pg[:], AF.Tanh,
                                     bias=bvt["bgh"][:, j:j + 1], scale=0.5)
                # u2 = (th + 1) * x = 2 * x * sigmoid(gate)
                nc.vector.scalar_tensor_tensor(
                    u2[jc][:], in0=th[:], scalar=1.0, in1=xsb[j][:, sl],
                    op0=OP.add, op1=OP.mult)

            def emit_po(j, qq):
                sl = slice(qq * 256, (qq + 1) * 256)
                po = psD.tile([P, 512], F32, tag="psD", name="psD")
                # bo-row first (start=True clears the bank), then accumulate
                nc.tensor.matmul(po[:, 0:256], borow[:, j * P:(j + 1) * P],
                                 onesrow[:, 0:256], start=True, stop=False)
                for i in range(ND):
                    nc.tensor.matmul(po[:, 0:256], wo[i][:, j * P:(j + 1) * P],
                                     ctxT[i][:, sl],
                                     start=False, stop=(i == ND - 1))
                fin = finp.tile([P, 256], F32, tag="fin", name="fin")
                u2s = u2[j * 2 + qq // 2][:, (qq % 2) * 256:(qq % 2) * 256 + 256]
                nc.vector.scalar_tensor_tensor(
                    fin[:], in0=u2s, scalar=0.5, in1=po[:, 0:256],
                    op0=OP.mult, op1=OP.add)
                nc.sync.dma_start(outT[j * P:(j + 1) * P, sl], fin[:])

            fillers = [(emit_gate, (jc,)) for jc in range(ND * 2)]
            ctxs = {}
            pend = []

            def emit_pv(qq, hp, K, es):
                if K == 0:
                    ctxs[(qq, hp)] = psC.tile([HD + 1, 512], F32,
                                              tag="psC", name="psC")
                ctx = ctxs[(qq, hp)]
                for si in range(2):
                    h = 2 * hp + si
                    for kt4 in range(4):
                        nc.tensor.matmul(
                            ctx[:, si * 256:(si + 1) * 256],
                            vsb[K * 4 + kt4][:, h * (HD + 1):(h + 1) * (HD + 1)],
                            es[si][:, kt4 * 256:(kt4 + 1) * 256],
                            start=(K == 0 and si == 0 and kt4 == 0),
                            stop=(K == 1 and si == 1 and kt4 == 3))
                if K == 0:
                    return
                # normalize: ctx rows 0..63 divided by the ones-row (row 64).
                # GpSimd cannot touch PSUM, so: ScalarE row copy -> DVE
                # reciprocal -> GpSimd broadcast -> DVE multiplies.
                ctx = ctxs.pop((qq, hp))
                rowt = normp.tile([1, 512], F32, tag="rowt", name="rowt")
                nc.scalar.copy(rowt[:], ctx[HD:HD + 1, :])
                rec = normp.tile([1, 512], F32, tag="rec", name="rec")
                nc.vector.reciprocal_approx_fast(rec[:], rowt[:])
                bc = normp.tile([HD, 512], F32, tag="bc", name="bc")
                nc.gpsimd.partition_broadcast(bc[:], rec[:])
                for si in range(2):
                    nc.vector.tensor_tensor(
                        ctxT[hp][si * HD:(si + 1) * HD, qq * 256:(qq + 1) * 256],
                        ctx[0:HD, si * 256:(si + 1) * 256],
                        bc[:, si * 256:(si + 1) * 256], OP.mult)
                if dbg is not None and qq == 0 and hp == 0:
                    ctxd = normp.tile([HD + 1, 512], F32, tag="ctxd",
                                      name="ctxd", bufs=1)
                    nc.vector.tensor_copy(ctxd[:], ctx[:])
                    nc.sync.dma_start(dbg["ctx00"], ctxd[:])
                    nc.sync.dma_start(dbg["rowt00"], rowt[:])
                    nc.sync.dma_start(dbg["rec00"], rec[:])
                    nc.sync.dma_start(dbg["bc00"], bc[:])
                if hp == 5:
                    for j in range(ND):
                        fillers.append((emit_po, (j, qq)))

            for qq in range(4):
                for hp in range(6):
                    nh = qq * 6 + hp
                    for K in range(2):
                        # one 4KB-per-partition-line DMA per (qq,hp,K):
                        # both heads of the pair
                        ebig = ebp.tile([P, 2048], BF16, tag="eb", name="eb")
                        nc.sync.dma_start(
                            ebig[:],
                            ebd[(nh * 2 + K) * P:(nh * 2 + K + 1) * P, :])
                        ebt = [ebig[:, si * N:(si + 1) * N]
                               for si in range(2)]
                        # PE fillers first: they run while ScalarE catches up
                        if fillers:
                            f, args = fillers.pop(0)
                            f(*args)
                        # qk matmuls: head pair on PE row groups 0/64
                        ss = [psS.tile([P, N], F32, tag="psS", name="psS")
                              for _ in range(2)]
                        for kt4 in range(4):
                            kt, off = K * 4 + kt4, kt4 * 256
                            for si in range(2):
                                rp = si * HD
                                nc.tensor.matmul(
                                    ss[si][:, off:off + 256],
                                    kT[hp][rp:rp + HD, kt * P:(kt + 1) * P],
                                    qT[hp][rp:rp + HD, qq * 256:(qq + 1) * 256],
                                    start=(kt4 % 2 == 0), stop=True)
                        es, ers = [], []
                        for si in range(2):
                            er = esp.tile([P, N], BF16, tag="esr", name="esr")
                            nc.scalar.activation(er[:], ss[si][:], AF.Exp)
                            e = esp.tile([P, N], BF16, tag="es", name="es",
                                         bufs=6)
                            # offload 1 in 4 multiplies to GpSimd (it is
                            # ~3.5x slower per element but otherwise idle)
                            eng = (nc.gpsimd if si == 1 and (hp + K) % 2 == 1
                                   else nc.vector)
                            eng.tensor_tensor(e[:], er[:], ebt[si], OP.mult)
                            es.append(e)
                            ers.append(er)
                        if dbg is not None and qq == 0 and hp == 0 and K == 0:
                            ssd = normp.tile([P, N], F32, tag="ssd",
                                             name="ssd", bufs=1)
                            nc.vector.tensor_copy(ssd[:], ss[0][:])
                            nc.sync.dma_start(dbg["ss00"], ssd[:])
                            nc.sync.dma_start(dbg["er00"], ers[0][:])
                            nc.sync.dma_start(dbg["es00"], es[0][:])
                            nc.sync.dma_start(dbg["eb00"], ebt[0])
                            esg = normp.tile([P, N], BF16, tag="esg",
                                             name="esg", bufs=1)
                            nc.gpsimd.tensor_tensor(esg[:], ers[0][:],
                                                    ebt[0], OP.mult)
                            nc.sync.dma_start(dbg["esg00"], esg[:])
                        # depth-2 software pipeline: PV trails by two epochs
                        # so the PE never waits on the exp->mult chain.
                        pend.append((qq, hp, K, es))
                        if len(pend) > 2:
                            emit_pv(*pend.pop(0))
            while pend:
                emit_pv(*pend.pop(0))
            # ---------------- Phase D tail ----------------
            for f, args in fillers:
                f(*args)
            if dbg is not None:
                nc.sync.dma_start(dbg["ctxT0"], ctxT[0][:])
                nc.sync.dma_start(dbg["u20"], u2[0][:])


_cache = {}


DBG_SHAPES = {
    "qT0": ([P, N], BF16), "kT0": ([P, N], BF16), "v0": ([P, VW], BF16),
    "eb00": ([P, N], BF16), "ss00": ([P, N], F32), "er00": ([P, N], BF16),
    "es00": ([P, N], BF16), "esg00": ([P, N], BF16),
    "ctx00": ([HD + 1, 512], F32), "rowt00": ([1, 512], F32),
    "rec00": ([1, 512], F32), "bc00": ([HD, 512], F32),
    "ctxT0": ([P, N], BF16), "u20": ([P, 512], BF16),
}


def _build(debug=False):
    key = ("nc", debug)
    if key in _cache:
        return _cache[key]
    nc = bacc.Bacc("TRN2", target_bir_lowering=False, debug=False, num_devices=8)
    xT = nc.dram_tensor("xT", [D, N], BF16, kind="ExternalInput")
    ebd = nc.dram_tensor("ebd", [48 * P, 2048], BF16, kind="ExternalInput")
    w = {nm: nc.dram_tensor(nm, [D, D], BF16, kind="ExternalInput")
         for nm in ("wq", "wk", "wv", "wg", "wo")}
    w["borow"] = nc.dram_tensor("borow", [1, D], BF16, kind="ExternalInput")
    bvec = {nm: nc.dram_tensor(nm, [D], F32, kind="ExternalInput")
            for nm in ("bq", "bk", "bgh")}
    outT = nc.dram_tensor("outT", [D, N], F32, kind="ExternalOutput")
    dbg = None
    if debug:
        dbg = {nm: nc.dram_tensor("dbg_" + nm, sh, dt,
                                  kind="ExternalOutput").ap()
               for nm, (sh, dt) in DBG_SHAPES.items()}
    with tile.TileContext(nc) as tc:
        _emit(nc, tc, xT.ap(), ebd.ap(), {k: v.ap() for k, v in w.items()},
              bvec, outT.ap(), dbg=dbg)
    nc.compile()
    _cache[key] = nc
    return nc


def _prep(inputs):
    scaling = HD ** (-0.5)
    shared = {
        "wq": _bf(inputs["Wq"].T * scaling),
        "wk": _bf(inputs["Wk"].T),
        "wv": _bf(inputs["Wv"].T),
        "wg": _bf(inputs["Wg"].T),
        "wo": _bf(inputs["Wo"].T),
        "borow": _bf(inputs["bo"] + inputs["Wo"] @ inputs["bv"]).reshape(1, D),
        "bq": np.ascontiguousarray(inputs["bq"] * scaling, np.float32),
        "bk": np.ascontiguousarray(inputs["bk"], np.float32),
        "bgh": np.ascontiguousarray(0.5 * inputs["bg"], np.float32),
    }
    ab = np.asarray(inputs["attn_bias"], np.float32)
    nd = np.asarray(inputs["ndata"], np.float32)
    in_maps = []
    for b in range(B):
        m = dict(shared)
        m["xT"] = _bf(nd[b].T)
        # exp(bias) tiled as the scores PSUM layout: one [128, (K si kt4 q)]
        # slab per (qq, hp) head-pair/query-block.
        e = np.exp(ab[b])                                  # [q, k, h]
        e = e.reshape(4, 256, 2, 4, P, ND, 2)              # qq qi K kt4 p hp si
        e = e.transpose(0, 5, 2, 4, 6, 3, 1)               # qq hp K p si kt4 qi
        m["ebd"] = np.ascontiguousarray(e.reshape(48 * P, 2048)).astype(
            ml_dtypes.bfloat16)
        in_maps.append(m)
    return in_maps


def run(inputs, trace=False, debug=False, **kw):
    nc = _build(debug=debug)
    in_maps = _prep(inputs)
    res = run_bass_kernel_spmd(nc, in_maps, core_ids=list(range(B)),
                               trace=trace, **kw)
    out = np.stack([np.ascontiguousarray(r["outT"].T) for r in res.results])
    return out, res


def kernel(**inputs):
    out, _ = run(inputs)
    return out


# revision 44
# speedup vs baseline: 2.1246x; 2.1246x over previous
"""BiasedMHA + GLU fused Trainium2 kernel (v2: exp-bias multiply).

Problem: out = GLU(x) + OutProj(MHA(x, attn_bias))  with
  B=8, N=1024, D=768, H=12, HD=64, fp32 inputs/outputs.

Strategy: data-parallel over batch across the 8 NeuronCores (one batch
element per core, no collectives). Everything in a "transposed"
[channel, token] layout so every GEMM contracts the partition dim.

Key changes vs v1 (286us):
  * The additive attention bias is applied as exp(s+b) = exp(s)*exp(b):
    the host precomputes exp(attn_bias) in bf16 tiled exactly like the
    scores PSUM layout [k, (kt4 q)]; after ScalarE exp of the raw qk
    scores, a single elementwise multiply (DVE for one head of the
    pair, GpSimd for the other) applies the bias. This removes the 768
    PE identity-matmuls + LDWEIGHTS (~90us of PE time) that v1 spent
    transposing/injecting the bias via the PE array.
  * All GEMMs run in bf16 (same 1 col/cycle PE rate as fp32r, but
    FWL-accelerated weight loads, half the DMA/SBUF, and 2x DVE modes).
  * K-halves accumulate into one PSUM ctx tile (K-inner loop): no
    partial-context eviction/re-inject round trip.
  * The GLU gate and out-proj GEMMs are interleaved into the attention
    epochs as PE fillers: attention is ScalarE(exp)-paced (~2us/epoch
    vs 1.3us of PE work), so phase-D work rides in the PE idle slots.
    bo is folded in via a [1,128] ones-row matmul into the same PSUM.

  Error budget: bf16 rounding of x/weights/q/k/v/exp adds ~5e-3
  relative error (vs 2e-2 tolerance), validated in numpy simulation.
"""

import os
import sys

for _p in ("/opt/trn_rl_repo", "/root/.axon_site/_ro/trn_rl_repo"):
    if os.path.isdir(_p) and _p not in sys.path:
        sys.path.insert(0, _p)

import numpy as np
import ml_dtypes

import concourse.bacc as bacc
import concourse.mybir as mybir
from concourse import tile
from concourse.bass_utils import run_bass_kernel_spmd

B, N, D, H, HD = 8, 1024, 768, 12, 64
P = 128
ND = D // P           # 6 channel tiles
NN = N // P           # 8 token tiles
VW = H * (HD + 1)     # 780: v layout [token, h*(64+1)] with ones column

F32 = mybir.dt.float32
BF16 = mybir.dt.bfloat16
AF = mybir.ActivationFunctionType
OP = mybir.AluOpType


def _bf(x):
    return np.ascontiguousarray(x, dtype=np.float32).astype(ml_dtypes.bfloat16)


def _emit(nc, tc, xT, ebd, w, bvec, outT, dbg=None):
    with tc.tile_pool(name="const", bufs=1) as constp, \
         tc.tile_pool(name="xp", bufs=1) as xp, \
         tc.tile_pool(name="qkvT", bufs=1) as qkvp, \
         tc.tile_pool(name="ctxTp", bufs=1) as ctxp, \
         tc.tile_pool(name="ebp", bufs=12) as ebp, \
         tc.tile_pool(name="esp", bufs=3) as esp, \
         tc.tile_pool(name="epi", bufs=1) as epip, \
         tc.tile_pool(name="wDE", bufs=1) as wde:

        # ---- constants + ScalarE exp-table warmup (load during phase B)
        dum = constp.tile([1, 16], F32, tag="dum", name="dum")
        nc.vector.memset(dum[:], 0.25)
        dum2 = constp.tile([1, 16], F32, tag="dum2", name="dum2")
        nc.scalar.activation(dum2[:], dum[:], AF.Exp)

        bvt = {}
        for nm in ("bq", "bk", "bgh"):
            t = constp.tile([P, ND], F32, tag=f"t{nm}", name=f"t{nm}")
            nc.sync.dma_start(t[:], bvec[nm].ap().rearrange("(j p) -> p j", p=P))
            bvt[nm] = t
        ones12 = constp.tile([P, H], BF16, tag="ones12", name="ones12")
        nc.vector.memset(ones12[:], 1.0)
        onesrow = constp.tile([1, N], BF16, tag="onesrow", name="onesrow")
        nc.vector.memset(onesrow[:], 1.0)

        xsb = [xp.tile([P, N], BF16, tag=f"x{i}", name=f"x{i}")
               for i in range(ND)]
        qT = [qkvp.tile([P, N], BF16, tag=f"qT{i}", name=f"qT{i}")
              for i in range(ND)]
        kT = [qkvp.tile([P, N], BF16, tag=f"kT{i}", name=f"kT{i}")
              for i in range(ND)]
        vsb = [qkvp.tile([P, VW], BF16, tag=f"v{t}", name=f"v{t}")
               for t in range(NN)]
        vv = [t.rearrange("p (h c) -> p h c", c=HD + 1) for t in vsb]
        ctxT = [ctxp.tile([P, N], BF16, tag=f"cT{i}", name=f"cT{i}")
                for i in range(ND)]
        u2 = [epip.tile([P, 512], BF16, tag=f"u2_{j}_{c}", name=f"u2_{j}_{c}")
              for j in range(ND) for c in range(2)]

        wo = [wde.tile([P, D], BF16, tag=f"wo{i}", name=f"wo{i}")
              for i in range(ND)]
        wg = [wde.tile([P, D], BF16, tag=f"wg{i}", name=f"wg{i}")
              for i in range(ND)]
        borow = wde.tile([1, D], BF16, tag="borow", name="borow")

        # ---------------- Phase B: q/k/v projections ----------------
        with tc.tile_pool(name="xTw", bufs=1) as xwp, \
             tc.tile_pool(name="psB", bufs=2, space="PSUM") as psB:
            wsb = {nm: [xwp.tile([P, D], BF16, tag="wpool",
                                 name=f"{nm}{i}", bufs=18)
                        for i in range(ND)]
                   for nm in ("wq", "wk", "wv")}
            # critical-path DMAs first, split so the j=0/c=0 GEMM group can
            # start after ~1.2MB instead of ~4.5MB.
            for i in range(ND):
                nc.sync.dma_start(wsb["wq"][i][:, 0:P],
                                  w["wq"][i * P:(i + 1) * P, 0:P])
                nc.sync.dma_start(xsb[i][:, 0:512],
                                  xT[i * P:(i + 1) * P, 0:512])
            for i in range(ND):
                nc.sync.dma_start(wsb["wq"][i][:, P:D],
                                  w["wq"][i * P:(i + 1) * P, P:D])
                nc.sync.dma_start(xsb[i][:, 512:N],
                                  xT[i * P:(i + 1) * P, 512:N])
            for i in range(ND):
                nc.sync.dma_start(wsb["wk"][i][:], w["wk"][i * P:(i + 1) * P, :])
            for i in range(ND):
                nc.sync.dma_start(wsb["wv"][i][:], w["wv"][i * P:(i + 1) * P, :])
            # phase C/D weights queued behind the phase-B critical path;
            # the attention-bias stream (emitted in phase C) follows them.
            for i in range(ND):
                nc.sync.dma_start(wg[i][:], w["wg"][i * P:(i + 1) * P, :])
            for i in range(ND):
                nc.sync.dma_start(wo[i][:], w["wo"][i * P:(i + 1) * P, :])
            nc.sync.dma_start(borow[:], w["borow"][0:1, :])

            # warmup burst: keeps the PE busy through the initial DMA wait
            # so the HAM clock-gate opens to 2.4 GHz before the real GEMMs.
            wup = constp.tile([P, 512], BF16, tag="wup", name="wup")
            nc.vector.memset(wup[:], 0.125)
            wps = psB.tile([P, 512], F32, tag="psB", name="psB")
            for i in range(12):
                nc.tensor.matmul(wps[:], wup[:, 0:P], wup[:],
                                 start=True, stop=True, skip_group_check=True)

            for nm, dst in (("wq", qT), ("wk", kT)):
                bt = bvt["bq" if nm == "wq" else "bk"]
                for j in range(ND):
                    for c in range(2):
                        ps = psB.tile([P, 512], F32, tag="psB", name="psB")
                        for i in range(ND):
                            nc.tensor.matmul(
                                ps[:], wsb[nm][i][:, j * P:(j + 1) * P],
                                xsb[i][:, c * 512:(c + 1) * 512],
                                start=(i == 0), stop=(i == ND - 1))
                        nc.scalar.activation(dst[j][:, c * 512:(c + 1) * 512],
                                             ps[:], AF.Identity,
                                             bias=bt[:, j:j + 1])

            for t in range(NN):
                nc.vector.tensor_copy(vv[t][:, :, HD], ones12[:])
                for c, (lo, sz) in enumerate(((0, 512), (512, 256))):
                    ps = psB.tile([P, 512], F32, tag="psB", name="psB")
                    for i in range(ND):
                        nc.tensor.matmul(
                            ps[:, 0:sz], xsb[i][:, t * P:(t + 1) * P],
                            wsb["wv"][i][:, lo:lo + sz],
                            start=(i == 0), stop=(i == ND - 1))
                    h0 = lo // HD
                    nc.vector.tensor_copy(
                        vv[t][:, h0:h0 + sz // HD, 0:HD],
                        ps[:, 0:sz].rearrange("p (h c) -> p h c", c=HD))

        if dbg is not None:
            nc.sync.dma_start(dbg["qT0"], qT[0][:])
            nc.sync.dma_start(dbg["kT0"], kT[0][:])
            nc.sync.dma_start(dbg["v0"], vsb[0][:])

        # ---------------- Phase C: attention + interleaved D fillers ----
        with tc.tile_pool(name="psS", bufs=2, space="PSUM") as psS, \
             tc.tile_pool(name="psC", bufs=2, space="PSUM") as psC, \
             tc.tile_pool(name="psD", bufs=2, space="PSUM") as psD, \
             tc.tile_pool(name="norm", bufs=2) as normp, \
             tc.tile_pool(name="finp", bufs=4) as finp:

            def emit_gate(jc):
                j, c = jc // 2, jc % 2
                sl = slice(c * 512, (c + 1) * 512)
                pg = psD.tile([P, 512], F32, tag="psD", name="psD")
                for i in range(ND):
                    nc.tensor.matmul(pg[:], wg[i][:, j * P:(j + 1) * P],
                                     xsb[i][:, sl],
                                     start=(i == 0), stop=(i == ND - 1))
                th = normp.tile([P, 512], BF16, tag="th", name="th", bufs=3)
                # tanh(0.5*g + 0.5*bg); bgh = 0.5*bg host-side
                nc.scalar.activation(th[:], pg[:], AF.Tanh,
                                     bias=bvt["bgh"][:, j:j + 1], scale=0.5)
                # u2 = (th + 1) * x = 2 * x * sigmoid(gate)
                nc.vector.scalar_tensor_tensor(
                    u2[jc][:], in0=th[:], scalar=1.0, in1=xsb[j][:, sl],
                    op0=OP.add, op1=OP.mult)

            def emit_po(j, qq):
                sl = slice(qq * 256, (qq + 1) * 256)
                po = psD.tile([P, 512], F32, tag="psD", name="psD")
                # bo-row first (start=True clears the bank), then accumulate
                nc.tensor.matmul(po[:, 0:256], borow[:, j * P:(j + 1) * P],
                                 onesrow[:, 0:256], start=True, stop=False)
                for i in range(ND):
                    nc.tensor.matmul(po[:, 0:256], wo[i][:, j * P:(j + 1) * P],
                                     ctxT[i][:, sl],
                                     start=False, stop=(i == ND - 1))
                fin = finp.tile([P, 256], F32, tag="fin", name="fin")
                u2s = u2[j * 2 + qq // 2][:, (qq % 2) * 256:(qq % 2) * 256 + 256]
                nc.vector.scalar_tensor_tensor(
                    fin[:], in0=u2s, scalar=0.5, in1=po[:, 0:256],
                    op0=OP.mult, op1=OP.add)
                nc.sync.dma_start(outT[j * P:(j + 1) * P, sl], fin[:])

            fillers = [(jc, emit_gate, (jc,)) for jc in range(ND * 2)]
            ctxs = {}
            pend = []
            ep_idx = 0

            def emit_pv(qq, hp, K, es):
                if K == 0:
                    ctxs[(qq, hp)] = psC.tile([HD + 1, 512], F32,
                                              tag="psC", name="psC")
                ctx = ctxs[(qq, hp)]
                for si in range(2):
                    h = 2 * hp + si
                    for kt4 in range(4):
                        nc.tensor.matmul(
                            ctx[:, si * 256:(si + 1) * 256],
                            vsb[K * 4 + kt4][:, h * (HD + 1):(h + 1) * (HD + 1)],
                            es[si][:, kt4 * 256:(kt4 + 1) * 256],
                            start=(K == 0 and si == 0 and kt4 == 0),
                            stop=(K == 1 and si == 1 and kt4 == 3))
                if K == 0:
                    return
                # normalize: ctx rows 0..63 divided by the ones-row (row 64).
                # GpSimd cannot touch PSUM, so: ScalarE row copy -> DVE
                # reciprocal -> GpSimd broadcast -> DVE multiplies.
                ctx = ctxs.pop((qq, hp))
                rowt = normp.tile([1, 512], F32, tag="rowt", name="rowt")
                nc.scalar.copy(rowt[:], ctx[HD:HD + 1, :])
                rec = normp.tile([1, 512], F32, tag="rec", name="rec")
                nc.vector.reciprocal_approx_fast(rec[:], rowt[:])
                bc = normp.tile([HD, 512], F32, tag="bc", name="bc")
                nc.gpsimd.partition_broadcast(bc[:], rec[:])
                for si in range(2):
                    nc.vector.tensor_tensor(
                        ctxT[hp][si * HD:(si + 1) * HD, qq * 256:(qq + 1) * 256],
                        ctx[0:HD, si * 256:(si + 1) * 256],
                        bc[:, si * 256:(si + 1) * 256], OP.mult)
                if dbg is not None and qq == 0 and hp == 0:
                    ctxd = normp.tile([HD + 1, 512], F32, tag="ctxd",
                                      name="ctxd", bufs=1)
                    nc.vector.tensor_copy(ctxd[:], ctx[:])
                    nc.sync.dma_start(dbg["ctx00"], ctxd[:])
                    nc.sync.dma_start(dbg["rowt00"], rowt[:])
                    nc.sync.dma_start(dbg["rec00"], rec[:])
                    nc.sync.dma_start(dbg["bc00"], bc[:])
                if hp == 5:
                    # gate on epoch (qq+1)*12+4+j: the PE must not reach the
                    # out-proj before its normalize chain (DVE+GpSimd) is done
                    for j in range(ND):
                        fillers.append(((qq + 1) * 12 + 4 + j,
                                        emit_po, (j, qq)))

            for qq in range(4):
                for hp in range(6):
                    nh = qq * 6 + hp
                    for K in range(2):
                        # one 4KB-per-partition-line DMA per (qq,hp,K):
                        # both heads of the pair
                        ebig = ebp.tile([P, 2048], BF16, tag="eb", name="eb")
                        nc.sync.dma_start(
                            ebig[:],
                            ebd[(nh * 2 + K) * P:(nh * 2 + K + 1) * P, :])
                        ebt = [ebig[:, si * N:(si + 1) * N]
                               for si in range(2)]
                        # PE fillers first: they run while ScalarE catches up
                        if fillers and fillers[0][0] <= ep_idx:
                            _, f, args = fillers.pop(0)
                            f(*args)
                        ep_idx += 1
                        # qk matmuls: head pair on PE row groups 0/64
                        ss = [psS.tile([P, N], F32, tag="psS", name="psS")
                              for _ in range(2)]
                        for kt4 in range(4):
                            kt, off = K * 4 + kt4, kt4 * 256
                            for si in range(2):
                                rp = si * HD
                                nc.tensor.matmul(
                                    ss[si][:, off:off + 256],
                                    kT[hp][rp:rp + HD, kt * P:(kt + 1) * P],
                                    qT[hp][rp:rp + HD, qq * 256:(qq + 1) * 256],
                                    start=(kt4 % 2 == 0), stop=True)
                        es, ers = [], []
                        for si in range(2):
                            er = esp.tile([P, N], BF16, tag="esr", name="esr")
                            nc.scalar.activation(er[:], ss[si][:], AF.Exp)
                            e = esp.tile([P, N], BF16, tag="es", name="es",
                                         bufs=6)
                            # GpSimd does ONLY partition_broadcast: mixing op
                            # types makes it reload its ucode lib every op.
                            nc.vector.tensor_tensor(e[:], er[:], ebt[si],
                                                    OP.mult)
                            es.append(e)
                            ers.append(er)
                        if dbg is not None and qq == 0 and hp == 0 and K == 0:
                            ssd = normp.tile([P, N], F32, tag="ssd",
                                             name="ssd", bufs=1)
                            nc.vector.tensor_copy(ssd[:], ss[0][:])
                            nc.sync.dma_start(dbg["ss00"], ssd[:])
                            nc.sync.dma_start(dbg["er00"], ers[0][:])
                            nc.sync.dma_start(dbg["es00"], es[0][:])
                            nc.sync.dma_start(dbg["eb00"], ebt[0])
                            esg = normp.tile([P, N], BF16, tag="esg",
                                             name="esg", bufs=1)
                            nc.gpsimd.tensor_tensor(esg[:], ers[0][:],
                                                    ebt[0], OP.mult)
                            nc.sync.dma_start(dbg["esg00"], esg[:])
                        # depth-2 software pipeline: PV trails by two epochs
                        # so the PE never waits on the exp->mult chain.
                        pend.append((qq, hp, K, es))
                        if len(pend) > 2:
                            emit_pv(*pend.pop(0))
            while pend:
                emit_pv(*pend.pop(0))
            # ---------------- Phase D tail ----------------
            for _, f, args in sorted(fillers):
                f(*args)
            if dbg is not None:
                nc.sync.dma_start(dbg["ctxT0"], ctxT[0][:])
                nc.sync.dma_start(dbg["u20"], u2[0][:])


_cache = {}


DBG_SHAPES = {
    "qT0": ([P, N], BF16), "kT0": ([P, N], BF16), "v0": ([P, VW], BF16),
    "eb00": ([P, N], BF16), "ss00": ([P, N], F32), "er00": ([P, N], BF16),
    "es00": ([P, N], BF16), "esg00": ([P, N], BF16),
    "ctx00": ([HD + 1, 512], F32), "rowt00": ([1, 512], F32),
    "rec00": ([1, 512], F32), "bc00": ([HD, 512], F32),
    "ctxT0": ([P, N], BF16), "u20": ([P, 512], BF16),
}


def _build(debug=False):
    key = ("nc", debug)
    if key in _cache:
        return _cache[key]
    nc = bacc.Bacc("TRN2", target_bir_lowering=False, debug=False, num_devices=8)
    xT = nc.dram_tensor("xT", [D, N], BF16, kind="ExternalInput")
    ebd = nc.dram_tensor("ebd", [48 * P, 2048], BF16, kind="ExternalInput")
    w = {nm: nc.dram_tensor(nm, [D, D], BF16, kind="ExternalInput")
         for nm in ("wq", "wk", "wv", "wg", "wo")}
    w["borow"] = nc.dram_tensor("borow", [1, D], BF16, kind="ExternalInput")
    bvec = {nm: nc.dram_tensor(nm, [D], F32, kind="ExternalInput")
            for nm in ("bq", "bk", "bgh")}
    outT = nc.dram_tensor("outT", [D, N], F32, kind="ExternalOutput")
    dbg = None
    if debug:
        dbg = {nm: nc.dram_tensor("dbg_" + nm, sh, dt,
                                  kind="ExternalOutput").ap()
               for nm, (sh, dt) in DBG_SHAPES.items()}
    with tile.TileContext(nc) as tc:
        _emit(nc, tc, xT.ap(), ebd.ap(), {k: v.ap() for k, v in w.items()},
              bvec, outT.ap(), dbg=dbg)
    nc.compile()
    _cache[key] = nc
    return nc


def _prep(inputs):
    scaling = HD ** (-0.5)
    shared = {
        "wq": _bf(inputs["Wq"].T * scaling),
        "wk": _bf(inputs["Wk"].T),
        "wv": _bf(inputs["Wv"].T),
        "wg": _bf(inputs["Wg"].T),
        "wo": _bf(inputs["Wo"].T),
        "borow": _bf(inputs["bo"] + inputs["Wo"] @ inputs["bv"]).reshape(1, D),
        "bq": np.ascontiguousarray(inputs["bq"] * scaling, np.float32),
        "bk": np.ascontiguousarray(inputs["bk"], np.float32),
        "bgh": np.ascontiguousarray(0.5 * inputs["bg"], np.float32),
    }
    ab = np.asarray(inputs["attn_bias"], np.float32)
    nd = np.asarray(inputs["ndata"], np.float32)
    in_maps = []
    for b in range(B):
        m = dict(shared)
        m["xT"] = _bf(nd[b].T)
        # exp(bias) tiled as the scores PSUM layout: one [128, (K si kt4 q)]
        # slab per (qq, hp) head-pair/query-block.
        e = np.exp(ab[b])                                  # [q, k, h]
        e = e.reshape(4, 256, 2, 4, P, ND, 2)              # qq qi K kt4 p hp si
        e = e.transpose(0, 5, 2, 4, 6, 3, 1)               # qq hp K p si kt4 qi
        m["ebd"] = np.ascontiguousarray(e.reshape(48 * P, 2048)).astype(
            ml_dtypes.bfloat16)
        in_maps.append(m)
    return in_maps


def run(inputs, trace=False, debug=False, **kw):
    nc = _build(debug=debug)
    in_maps = _prep(inputs)
    res = run_bass_kernel_spmd(nc, in_maps, core_ids=list(range(B)),
                               trace=trace, **kw)
    out = np.stack([np.ascontiguousarray(r["outT"].T) for r in res.results])
    return out, res


def kernel(**inputs):
    out, _ = run(inputs)
    return out


# revision 47
# speedup vs baseline: 2.1527x; 1.0132x over previous
"""BiasedMHA + GLU fused Trainium2 kernel (v2: exp-bias multiply).

Problem: out = GLU(x) + OutProj(MHA(x, attn_bias))  with
  B=8, N=1024, D=768, H=12, HD=64, fp32 inputs/outputs.

Strategy: data-parallel over batch across the 8 NeuronCores (one batch
element per core, no collectives). Everything in a "transposed"
[channel, token] layout so every GEMM contracts the partition dim.

Key changes vs v1 (286us):
  * The additive attention bias is applied as exp(s+b) = exp(s)*exp(b):
    the host precomputes exp(attn_bias) in bf16 tiled exactly like the
    scores PSUM layout [k, (kt4 q)]; after ScalarE exp of the raw qk
    scores, a single elementwise multiply (DVE for one head of the
    pair, GpSimd for the other) applies the bias. This removes the 768
    PE identity-matmuls + LDWEIGHTS (~90us of PE time) that v1 spent
    transposing/injecting the bias via the PE array.
  * All GEMMs run in bf16 (same 1 col/cycle PE rate as fp32r, but
    FWL-accelerated weight loads, half the DMA/SBUF, and 2x DVE modes).
  * K-halves accumulate into one PSUM ctx tile (K-inner loop): no
    partial-context eviction/re-inject round trip.
  * The GLU gate and out-proj GEMMs are interleaved into the attention
    epochs as PE fillers: attention is ScalarE(exp)-paced (~2us/epoch
    vs 1.3us of PE work), so phase-D work rides in the PE idle slots.
    bo is folded in via a [1,128] ones-row matmul into the same PSUM.

  Error budget: bf16 rounding of x/weights/q/k/v/exp adds ~5e-3
  relative error (vs 2e-2 tolerance), validated in numpy simulation.
"""

import os
import sys

for _p in ("/opt/trn_rl_repo", "/root/.axon_site/_ro/trn_rl_repo"):
    if os.path.isdir(_p) and _p not in sys.path:
        sys.path.insert(0, _p)

import numpy as np
import ml_dtypes

import concourse.bacc as bacc
import concourse.mybir as mybir
from concourse import tile
from concourse.bass_utils import run_bass_kernel_spmd

B, N, D, H, HD = 8, 1024, 768, 12, 64
P = 128
ND = D // P           # 6 channel tiles
NN = N // P           # 8 token tiles
VW = H * (HD + 1)     # 780: v layout [token, h*(64+1)] with ones column

F32 = mybir.dt.float32
BF16 = mybir.dt.bfloat16
AF = mybir.ActivationFunctionType
OP = mybir.AluOpType


def _bf(x):
    return np.ascontiguousarray(x, dtype=np.float32).astype(ml_dtypes.bfloat16)


def _emit(nc, tc, xT, ebd, w, bvec, outT, dbg=None):
    with tc.tile_pool(name="const", bufs=1) as constp, \
         tc.tile_pool(name="xp", bufs=1) as xp, \
         tc.tile_pool(name="qkvT", bufs=1) as qkvp, \
         tc.tile_pool(name="ctxTp", bufs=1) as ctxp, \
         tc.tile_pool(name="ebp", bufs=12) as ebp, \
         tc.tile_pool(name="esp", bufs=3) as esp, \
         tc.tile_pool(name="epi", bufs=1) as epip, \
         tc.tile_pool(name="wDE", bufs=1) as wde:

        # ---- constants + ScalarE exp-table warmup (load during phase B)
        wup = constp.tile([P, 512], BF16, tag="wup", name="wup")
        nc.vector.memset(wup[:], 0.125)
        dum = constp.tile([1, 16], F32, tag="dum", name="dum")
        nc.vector.memset(dum[:], 0.25)
        dum2 = constp.tile([1, 16], F32, tag="dum2", name="dum2")
        nc.scalar.activation(dum2[:], dum[:], AF.Exp)

        bvt = {}
        for nm in ("bq", "bk", "bgh"):
            t = constp.tile([P, ND], F32, tag=f"t{nm}", name=f"t{nm}")
            nc.sync.dma_start(t[:], bvec[nm].ap().rearrange("(j p) -> p j", p=P))
            bvt[nm] = t
        ones12 = constp.tile([P, H], BF16, tag="ones12", name="ones12")
        nc.vector.memset(ones12[:], 1.0)
        onesrow = constp.tile([1, N], BF16, tag="onesrow", name="onesrow")
        nc.vector.memset(onesrow[:], 1.0)

        xsb = [xp.tile([P, N], BF16, tag=f"x{i}", name=f"x{i}")
               for i in range(ND)]
        qT = [qkvp.tile([P, N], BF16, tag=f"qT{i}", name=f"qT{i}")
              for i in range(ND)]
        kT = [qkvp.tile([P, N], BF16, tag=f"kT{i}", name=f"kT{i}")
              for i in range(ND)]
        vsb = [qkvp.tile([P, VW], BF16, tag=f"v{t}", name=f"v{t}")
               for t in range(NN)]
        vv = [t.rearrange("p (h c) -> p h c", c=HD + 1) for t in vsb]
        ctxT = [ctxp.tile([P, N], BF16, tag=f"cT{i}", name=f"cT{i}")
                for i in range(ND)]
        u2 = [epip.tile([P, 512], BF16, tag=f"u2_{j}_{c}", name=f"u2_{j}_{c}")
              for j in range(ND) for c in range(2)]

        wo = [wde.tile([P, D], BF16, tag=f"wo{i}", name=f"wo{i}")
              for i in range(ND)]
        wg = [wde.tile([P, D], BF16, tag=f"wg{i}", name=f"wg{i}")
              for i in range(ND)]
        borow = wde.tile([1, D], BF16, tag="borow", name="borow")

        # ---------------- Phase B: q/k/v projections ----------------
        with tc.tile_pool(name="xTw", bufs=1) as xwp, \
             tc.tile_pool(name="psB", bufs=2, space="PSUM") as psB:
            wsb = {nm: [xwp.tile([P, D], BF16, tag="wpool",
                                 name=f"{nm}{i}", bufs=18)
                        for i in range(ND)]
                   for nm in ("wq", "wk", "wv")}
            # critical-path DMAs: v-projection runs first (its DVE copies
            # then drain during the q/k GEMMs, so phase C starts with an
            # empty DVE queue), so x + wv come first.
            for i in range(ND):
                nc.sync.dma_start(xsb[i][:], xT[i * P:(i + 1) * P, :])
                nc.sync.dma_start(wsb["wv"][i][:], w["wv"][i * P:(i + 1) * P, :])
            for i in range(ND):
                nc.sync.dma_start(wsb["wq"][i][:], w["wq"][i * P:(i + 1) * P, :])
            for i in range(ND):
                nc.sync.dma_start(wsb["wk"][i][:], w["wk"][i * P:(i + 1) * P, :])
            # phase C/D weights queued behind the phase-B critical path;
            # the attention-bias stream (emitted in phase C) follows them.
            for i in range(ND):
                nc.sync.dma_start(wg[i][:], w["wg"][i * P:(i + 1) * P, :])
            for i in range(ND):
                nc.sync.dma_start(wo[i][:], w["wo"][i * P:(i + 1) * P, :])
            nc.sync.dma_start(borow[:], w["borow"][0:1, :])

            # warmup burst: keeps the PE busy through the initial DMA wait
            # so the HAM clock-gate opens to 2.4 GHz before the real GEMMs.
            wps = psB.tile([P, 512], F32, tag="psB", name="psB")
            for i in range(14):
                nc.tensor.matmul(wps[:], wup[:, 0:P], wup[:],
                                 start=True, stop=True, skip_group_check=True)

            for t in range(NN):
                nc.vector.tensor_copy(vv[t][:, :, HD], ones12[:])
                for c, (lo, sz) in enumerate(((0, 512), (512, 256))):
                    ps = psB.tile([P, 512], F32, tag="psB", name="psB")
                    for i in range(ND):
                        nc.tensor.matmul(
                            ps[:, 0:sz], xsb[i][:, t * P:(t + 1) * P],
                            wsb["wv"][i][:, lo:lo + sz],
                            start=(i == 0), stop=(i == ND - 1))
                    h0 = lo // HD
                    nc.vector.tensor_copy(
                        vv[t][:, h0:h0 + sz // HD, 0:HD],
                        ps[:, 0:sz].rearrange("p (h c) -> p h c", c=HD))

            for nm, dst in (("wq", qT), ("wk", kT)):
                bt = bvt["bq" if nm == "wq" else "bk"]
                for j in range(ND):
                    for c in range(2):
                        ps = psB.tile([P, 512], F32, tag="psB", name="psB")
                        for i in range(ND):
                            nc.tensor.matmul(
                                ps[:], wsb[nm][i][:, j * P:(j + 1) * P],
                                xsb[i][:, c * 512:(c + 1) * 512],
                                start=(i == 0), stop=(i == ND - 1))
                        nc.scalar.activation(dst[j][:, c * 512:(c + 1) * 512],
                                             ps[:], AF.Identity,
                                             bias=bt[:, j:j + 1])

        if dbg is not None:
            nc.sync.dma_start(dbg["qT0"], qT[0][:])
            nc.sync.dma_start(dbg["kT0"], kT[0][:])
            nc.sync.dma_start(dbg["v0"], vsb[0][:])

        # ---------------- Phase C: attention + interleaved D fillers ----
        with tc.tile_pool(name="psS", bufs=2, space="PSUM") as psS, \
             tc.tile_pool(name="psC", bufs=2, space="PSUM") as psC, \
             tc.tile_pool(name="psD", bufs=2, space="PSUM") as psD, \
             tc.tile_pool(name="norm", bufs=2) as normp, \
             tc.tile_pool(name="finp", bufs=4) as finp:

            def emit_gate(jc):
                j, c = jc // 2, jc % 2
                sl = slice(c * 512, (c + 1) * 512)
                pg = psD.tile([P, 512], F32, tag="psD", name="psD")
                for i in range(ND):
                    nc.tensor.matmul(pg[:], wg[i][:, j * P:(j + 1) * P],
                                     xsb[i][:, sl],
                                     start=(i == 0), stop=(i == ND - 1))
                th = normp.tile([P, 512], BF16, tag="th", name="th", bufs=3)
                # tanh(0.5*g + 0.5*bg); bgh = 0.5*bg host-side
                nc.scalar.activation(th[:], pg[:], AF.Tanh,
                                     bias=bvt["bgh"][:, j:j + 1], scale=0.5)
                # u2 = (th + 1) * x = 2 * x * sigmoid(gate)
                nc.vector.scalar_tensor_tensor(
                    u2[jc][:], in0=th[:], scalar=1.0, in1=xsb[j][:, sl],
                    op0=OP.add, op1=OP.mult)

            def emit_po(j, qq):
                sl = slice(qq * 256, (qq + 1) * 256)
                po = psD.tile([P, 512], F32, tag="psD", name="psD")
                # bo-row first (start=True clears the bank), then accumulate
                nc.tensor.matmul(po[:, 0:256], borow[:, j * P:(j + 1) * P],
                                 onesrow[:, 0:256], start=True, stop=False)
                for i in range(ND):
                    nc.tensor.matmul(po[:, 0:256], wo[i][:, j * P:(j + 1) * P],
                                     ctxT[i][:, sl],
                                     start=False, stop=(i == ND - 1))
                fin = finp.tile([P, 256], F32, tag="fin", name="fin")
                u2s = u2[j * 2 + qq // 2][:, (qq % 2) * 256:(qq % 2) * 256 + 256]
                nc.vector.scalar_tensor_tensor(
                    fin[:], in0=u2s, scalar=0.5, in1=po[:, 0:256],
                    op0=OP.mult, op1=OP.add)
                nc.sync.dma_start(outT[j * P:(j + 1) * P, sl], fin[:])

            fillers = [(jc, emit_gate, (jc,)) for jc in range(ND * 2)]
            ctxs = {}
            pend = []
            ep_idx = 0

            def emit_pv(qq, hp, K, es):
                if K == 0:
                    ctxs[(qq, hp)] = psC.tile([HD + 1, 512], F32,
                                              tag="psC", name="psC")
                ctx = ctxs[(qq, hp)]
                for si in range(2):
                    h = 2 * hp + si
                    for kt4 in range(4):
                        nc.tensor.matmul(
                            ctx[:, si * 256:(si + 1) * 256],
                            vsb[K * 4 + kt4][:, h * (HD + 1):(h + 1) * (HD + 1)],
                            es[si][:, kt4 * 256:(kt4 + 1) * 256],
                            start=(K == 0 and si == 0 and kt4 == 0),
                            stop=(K == 1 and si == 1 and kt4 == 3))
                if K == 0:
                    return
                # normalize: ctx rows 0..63 divided by the ones-row (row 64).
                # GpSimd cannot touch PSUM, so: ScalarE row copy -> DVE
                # reciprocal -> GpSimd broadcast -> DVE multiplies.
                ctx = ctxs.pop((qq, hp))
                rowt = normp.tile([1, 512], F32, tag="rowt", name="rowt")
                # alternate the row extraction between ScalarE and DVE to
                # balance the two C-phase pacer engines
                if hp % 2 == 0:
                    nc.scalar.copy(rowt[:], ctx[HD:HD + 1, :])
                else:
                    nc.vector.tensor_copy(rowt[:], ctx[HD:HD + 1, :])
                rec = normp.tile([1, 512], F32, tag="rec", name="rec")
                nc.vector.reciprocal_approx_fast(rec[:], rowt[:])
                bc = normp.tile([HD, 512], F32, tag="bc", name="bc")
                nc.gpsimd.partition_broadcast(bc[:], rec[:])
                for si in range(2):
                    nc.vector.tensor_tensor(
                        ctxT[hp][si * HD:(si + 1) * HD, qq * 256:(qq + 1) * 256],
                        ctx[0:HD, si * 256:(si + 1) * 256],
                        bc[:, si * 256:(si + 1) * 256], OP.mult)
                if dbg is not None and qq == 0 and hp == 0:
                    ctxd = normp.tile([HD + 1, 512], F32, tag="ctxd",
                                      name="ctxd", bufs=1)
                    nc.vector.tensor_copy(ctxd[:], ctx[:])
                    nc.sync.dma_start(dbg["ctx00"], ctxd[:])
                    nc.sync.dma_start(dbg["rowt00"], rowt[:])
                    nc.sync.dma_start(dbg["rec00"], rec[:])
                    nc.sync.dma_start(dbg["bc00"], bc[:])
                if hp == 5:
                    # gate on epoch (qq+1)*12+4+j: the PE must not reach the
                    # out-proj before its normalize chain (DVE+GpSimd) is done
                    for j in range(ND):
                        fillers.append(((qq + 1) * 12 + 4 + j,
                                        emit_po, (j, qq)))

            for qq in range(4):
                for hp in range(6):
                    nh = qq * 6 + hp
                    for K in range(2):
                        # one 4KB-per-partition-line DMA per (qq,hp,K):
                        # both heads of the pair
                        ebig = ebp.tile([P, 2048], BF16, tag="eb", name="eb")
                        nc.sync.dma_start(
                            ebig[:],
                            ebd[(nh * 2 + K) * P:(nh * 2 + K + 1) * P, :])
                        ebt = [ebig[:, si * N:(si + 1) * N]
                               for si in range(2)]
                        # PE fillers first: they run while ScalarE catches up
                        if fillers and fillers[0][0] <= ep_idx:
                            _, f, args = fillers.pop(0)
                            f(*args)
                        ep_idx += 1
                        # qk matmuls: head pair on PE row groups 0/64
                        ss = [psS.tile([P, N], F32, tag="psS", name="psS")
                              for _ in range(2)]
                        for kt4 in range(4):
                            kt, off = K * 4 + kt4, kt4 * 256
                            for si in range(2):
                                rp = si * HD
                                nc.tensor.matmul(
                                    ss[si][:, off:off + 256],
                                    kT[hp][rp:rp + HD, kt * P:(kt + 1) * P],
                                    qT[hp][rp:rp + HD, qq * 256:(qq + 1) * 256],
                                    start=(kt4 % 2 == 0), stop=True)
                        es, ers = [], []
                        for si in range(2):
                            er = esp.tile([P, N], BF16, tag="esr", name="esr")
                            nc.scalar.activation(er[:], ss[si][:], AF.Exp)
                            e = esp.tile([P, N], BF16, tag="es", name="es",
                                         bufs=6)
                            # GpSimd does ONLY partition_broadcast: mixing op
                            # types makes it reload its ucode lib every op.
                            nc.vector.tensor_tensor(e[:], er[:], ebt[si],
                                                    OP.mult)
                            es.append(e)
                            ers.append(er)
                        if dbg is not None and qq == 0 and hp == 0 and K == 0:
                            ssd = normp.tile([P, N], F32, tag="ssd",
                                             name="ssd", bufs=1)
                            nc.vector.tensor_copy(ssd[:], ss[0][:])
                            nc.sync.dma_start(dbg["ss00"], ssd[:])
                            nc.sync.dma_start(dbg["er00"], ers[0][:])
                            nc.sync.dma_start(dbg["es00"], es[0][:])
                            nc.sync.dma_start(dbg["eb00"], ebt[0])
                            esg = normp.tile([P, N], BF16, tag="esg",
                                             name="esg", bufs=1)
                            nc.gpsimd.tensor_tensor(esg[:], ers[0][:],
                                                    ebt[0], OP.mult)
                            nc.sync.dma_start(dbg["esg00"], esg[:])
                        # depth-2 software pipeline: PV trails by two epochs
                        # so the PE never waits on the exp->mult chain.
                        pend.append((qq, hp, K, es))
                        if len(pend) > 2:
                            emit_pv(*pend.pop(0))
            while pend:
                emit_pv(*pend.pop(0))
            # ---------------- Phase D tail ----------------
            for _, f, args in sorted(fillers):
                f(*args)
            if dbg is not None:
                nc.sync.dma_start(dbg["ctxT0"], ctxT[0][:])
                nc.sync.dma_start(dbg["u20"], u2[0][:])


_cache = {}


DBG_SHAPES = {
    "qT0": ([P, N], BF16), "kT0": ([P, N], BF16), "v0": ([P, VW], BF16),
    "eb00": ([P, N], BF16), "ss00": ([P, N], F32), "er00": ([P, N], BF16),
    "es00": ([P, N], BF16), "esg00": ([P, N], BF16),
    "ctx00": ([HD + 1, 512], F32), "rowt00": ([1, 512], F32),
    "rec00": ([1, 512], F32), "bc00": ([HD, 512], F32),
    "ctxT0": ([P, N], BF16), "u20": ([P, 512], BF16),
}


def _build(debug=False):
    key = ("nc", debug)
    if key in _cache:
        return _cache[key]
    nc = bacc.Bacc("TRN2", target_bir_lowering=False, debug=False, num_devices=8)
    xT = nc.dram_tensor("xT", [D, N], BF16, kind="ExternalInput")
    ebd = nc.dram_tensor("ebd", [48 * P, 2048], BF16, kind="ExternalInput")
    w = {nm: nc.dram_tensor(nm, [D, D], BF16, kind="ExternalInput")
         for nm in ("wq", "wk", "wv", "wg", "wo")}
    w["borow"] = nc.dram_tensor("borow", [1, D], BF16, kind="ExternalInput")
    bvec = {nm: nc.dram_tensor(nm, [D], F32, kind="ExternalInput")
            for nm in ("bq", "bk", "bgh")}
    outT = nc.dram_tensor("outT", [D, N], F32, kind="ExternalOutput")
    dbg = None
    if debug:
        dbg = {nm: nc.dram_tensor("dbg_" + nm, sh, dt,
                                  kind="ExternalOutput").ap()
               for nm, (sh, dt) in DBG_SHAPES.items()}
    with tile.TileContext(nc) as tc:
        _emit(nc, tc, xT.ap(), ebd.ap(), {k: v.ap() for k, v in w.items()},
              bvec, outT.ap(), dbg=dbg)
    nc.compile()
    _cache[key] = nc
    return nc


def _prep(inputs):
    scaling = HD ** (-0.5)
    shared = {
        "wq": _bf(inputs["Wq"].T * scaling),
        "wk": _bf(inputs["Wk"].T),
        "wv": _bf(inputs["Wv"].T),
        "wg": _bf(inputs["Wg"].T),
        "wo": _bf(inputs["Wo"].T),
        "borow": _bf(inputs["bo"] + inputs["Wo"] @ inputs["bv"]).reshape(1, D),
        "bq": np.ascontiguousarray(inputs["bq"] * scaling, np.float32),
        "bk": np.ascontiguousarray(inputs["bk"], np.float32),
        "bgh": np.ascontiguousarray(0.5 * inputs["bg"], np.float32),
    }
    ab = np.asarray(inputs["attn_bias"], np.float32)
    nd = np.asarray(inputs["ndata"], np.float32)
    in_maps = []
    for b in range(B):
        m = dict(shared)
        m["xT"] = _bf(nd[b].T)
        # exp(bias) tiled as the scores PSUM layout: one [128, (K si kt4 q)]
        # slab per (qq, hp) head-pair/query-block.
        e = np.exp(ab[b])                                  # [q, k, h]
        e = e.reshape(4, 256, 2, 4, P, ND, 2)              # qq qi K kt4 p hp si
        e = e.transpose(0, 5, 2, 4, 6, 3, 1)               # qq hp K p si kt4 qi
        m["ebd"] = np.ascontiguousarray(e.reshape(48 * P, 2048)).astype(
            ml_dtypes.bfloat16)
        in_maps.append(m)
    return in_maps


def run(inputs, trace=False, debug=False, **kw):
    nc = _build(debug=debug)
    in_maps = _prep(inputs)
    res = run_bass_kernel_spmd(nc, in_maps, core_ids=list(range(B)),
                               trace=trace, **kw)
    out = np.stack([np.ascontiguousarray(r["outT"].T) for r in res.results])
    return out, res


def kernel(**inputs):
    out, _ = run(inputs)
    return out
